# revision 12
# baseline (speedup 1.0000x reference)
"""Trainium2 Bass kernel for nn_EnsembleSpace (moe_routing).

Reference computation (B=128, E=64, D1=512, D2=2048):
    idx  = top_k(config, 8)                     # [B, E] routing logits
    cfg  = softmax(config * topk_mask)          # full-width softmax
    cfg  = where(cfg < 1e-4, 0, cfg)
    out  = cfg @ kernel.reshape(E, D1*D2)       # [B, D1*D2] -> [B, D1, D2]

The problem is memory-bound: the expert table is read once and the output
written once, and the per-core HBM limit is ~358 GB/s.  Two levers:

1. D1-sharding over the 8 cores (each core reads 1/8 of the table and
   writes 1/8 of the output, no collective).
2. 16-bit streaming: the table slice is pre-cast to bf16 on the host
   while laying out the shards, and the output is written to HBM as fp16
   and upcast on the host during the gather.  This halves per-core
   traffic from 96 MB to 48 MB.  Matmuls run bf16 x bf16 -> fp32 PSUM
   (bf16 streams 1 column/cycle on the PE; fp16 risks the half-rate
   mantissa path), so the only precision loss is quantization of the
   table / weights / output (~4e-3 rel err, inside the 2e-2 gate with
   4.6x margin).

Each core:
  1. computes the routing weights cfg [128, 64] on-chip (iterative top-8
     via 7 max+knockout rounds, exp+sum via one ACT op, eps mask),
  2. transposes cfg to [E, B] via two col-tiled identity matmuls so the
     weights land in BOTH partition halves (rows 0-63 and 64-127), then
     converts them to bf16,
  3. streams its table slice as 16 tiles of [128, 4096] bf16 (8 KB per
     partition line, full SBUF-port rate); each tile holds 4 D1-rows
     (two in each partition half); matmuls (N=512, bf16) alternate
     between the two array row-halves every instruction so each
     LDWEIGHTS targets the row-group the running matmul does NOT occupy
     and pulls ahead (same-half back-to-back matmuls serialize
     LDW+fill+drain at ~750 ns; alternated they stream at ~1 col/cycle).
     Pairs of [128, 1024] 2-bank PSUM tiles (4 bufs) drain via
     PSUM->SBUF copies (alternating DVE / ACT) that downconvert into
     fp16 out tiles; 1 MB fp16 out DMAs per D1-row pair.

Input DMAs and 1/4 of output DMAs ride the SP HWDGE ring, the other
output DMAs the ACT ring (~24 MB per ring); the 16 SDMA engines drain
both rings round-robin at packet granularity.
"""

import sys

for _p in ("/opt/trn_rl_repo", "/root/.axon_site/_ro/trn_rl_repo"):
    if _p not in sys.path:
        sys.path.append(_p)

import ml_dtypes
import numpy as np
import concourse.bass as bass
from concourse import tile, masks, bass_utils

mybir = bass.mybir
_f32 = mybir.dt.float32
_f16 = mybir.dt.float16
_bf16 = mybir.dt.bfloat16
_X = mybir.AxisListType.X
_alu = mybir.AluOpType

B, E, D1, D2 = 128, 64, 512, 2048
N_CORES = 8
D1_SH = D1 // N_CORES          # 64 D1-rows per core
ROWS_PER_TILE = 4              # D1-rows per [128, 4096] bf16 input tile
N_TILES = D1_SH // ROWS_PER_TILE   # 16
TW = 2 * D2                    # tile free size: 4096 x 16-bit = 8 KB/partition
MM_N = 512                     # one bf16 matmul per PSUM-bank-aligned slice
TOP_K = 8
SPARSE_EPS = 1e-4

_TRACE = False                 # test.py flips this for profiled runs
_TRACE_KWARGS = {}
LAST_RESULT = None             # BassKernelResults of the last run


def _split_multi_waits(nc):
    """This walrus build rejects >1 sync-wait per instruction.  Tile's
    add_semaphores emits multi-wait instructions (and the kernel-tail drain
    waits on every live semaphore).  Move the extra waits onto same-engine
    nops inserted immediately before the instruction — the engine executes
    serially, so blocking on the nops is equivalent."""
    n_split = 0
    for bb in nc.m.functions[0].blocks:
        out = []
        changed = False
        for inst in bb.instructions:
            si = inst.sync_info
            waits = list(si.on_wait) if (si is not None and si.on_wait) else []
            if len(waits) > 1:
                changed = True
                for w in waits[:-1]:
                    n_split += 1
                    nop = mybir.InstNoOp(name=f"I-waitsplit-{n_split}")
                    nop.engine = inst.engine
                    nop.sync_info = mybir.SyncInfo(on_wait=[w], on_update=[])
                    out.append(nop)
                inst.sync_info = mybir.SyncInfo(
                    on_wait=[waits[-1]], on_update=list(si.on_update or [])
                )
            out.append(inst)
        if changed:
            bb.instructions = out


def _routing_weights(nc, rp, pp, cfg_ap):
    """cfg [B, E] -> cfgT [E, B] bf16 in SBUF (top-8 mask, softmax, eps)."""
    cfgin = rp.tile([B, E], _f32, tag="cfgin")
    nc.sync.dma_start(cfgin[:], cfg_ap[:])

    # 8th-largest per row, in exp-space: exp(config) is positive and
    # order-preserving, so "knock out the max" is a 2-op zero-replace
    # (zero can never shadow a remaining value) instead of a 3-op -inf add
    e0 = rp.tile([B, E], _f32, tag="e0")
    nc.scalar.activation(e0[:], cfgin[:], mybir.ActivationFunctionType.Exp)
    t = rp.tile([B, E], _f32, tag="t")
    nc.vector.tensor_copy(t[:], e0[:])
    mk = rp.tile([B, 1], _f32, tag="mk")
    for _ in range(TOP_K - 1):
        nc.vector.reduce_max(mk[:], t[:], axis=_X)
        nc.vector.scalar_tensor_tensor(
            t[:], t[:], mk[:], t[:], op0=_alu.is_lt, op1=_alu.mult
        )
    m8 = rp.tile([B, 1], _f32, tag="m8")
    nc.vector.reduce_max(m8[:], t[:], axis=_X)

    # cfg0 = (exp(config) >= exp(m8)) * config ; softmax ; eps mask
    cfg0 = rp.tile([B, E], _f32, tag="cfg0")
    nc.vector.scalar_tensor_tensor(
        cfg0[:], e0[:], m8[:], cfgin[:], op0=_alu.is_ge, op1=_alu.mult
    )
    ecfg = rp.tile([B, E], _f32, tag="ecfg")
    zs = rp.tile([B, 1], _f32, tag="zs")
    nc.scalar.activation(
        ecfg[:], cfg0[:], mybir.ActivationFunctionType.Exp, accum_out=zs[:]
    )
    rz = rp.tile([B, 1], _f32, tag="rz")
    nc.vector.reciprocal(rz[:], zs[:])
    cfgn = rp.tile([B, E], _f32, tag="cfgn")
    nc.vector.tensor_scalar_mul(cfgn[:], ecfg[:], rz[:])
    cfgf = rp.tile([B, E], _f32, tag="cfgf")
    nc.vector.scalar_tensor_tensor(
        cfgf[:], cfgn[:], SPARSE_EPS, cfgn[:], op0=_alu.is_ge, op1=_alu.mult
    )

    # transpose to [E, B], replicated into both partition halves so the
    # row-packed matmuls can source weights at array rows 0-63 and 64-127;
    # the PSUM->SBUF copy downconverts to bf16 for the bf16 matmuls
    ident = rp.tile([B, B], _f32, tag="ident")
    masks.make_identity(nc, ident[:])
    psT = pp.tile([B, B], _f32, tag="ps")
    nc.tensor.matmul(psT[0:E, :], cfgf[:], ident[:], start=True, stop=True)
    nc.tensor.matmul(psT[E:2 * E, :], cfgf[:], ident[:], start=True, stop=True)
    cfgT2 = rp.tile([B, B], _bf16, tag="cfgT2")
    nc.vector.tensor_copy(cfgT2[:], psT[:])
    return cfgT2


def _build():
    nc = bass.Bass(
        "TRN2", target_bir_lowering=False, debug=False, num_devices=N_CORES
    )
    cfg_ap = nc.dram_tensor("config", [B, E], _f32, kind="ExternalInput").ap()
    ks_ap = nc.dram_tensor(
        "kslice", [N_TILES, B, TW], _bf16, kind="ExternalInput"
    ).ap()
    out_ap = nc.dram_tensor(
        "out", [2 * N_TILES, B, TW], _f16, kind="ExternalOutput"
    ).ap()

    with tile.TileContext(nc) as tc:
        with tc.tile_pool(name="route", bufs=1) as rp, \
             tc.tile_pool(name="inp", bufs=12) as ip, \
             tc.tile_pool(name="outp", bufs=10) as op_, \
             tc.tile_pool(name="ps", bufs=4, space="PSUM") as pp:
            cfgT2 = _routing_weights(nc, rp, pp, cfg_ap)
            out_dma_i = 0
            GW = 1024              # psum group width: 2 banks
            for t in range(N_TILES):
                kt = ip.tile([B, TW], _bf16, tag="kt")
                nc.sync.dma_start(kt[:], ks_ap[t])
                otA = op_.tile([B, TW], _f16, tag="ot")
                otB = op_.tile([B, TW], _f16, tag="ot")
                # 4 psum groups per tile; group g covers free slice
                # [foff, foff+1024) of BOTH out tiles.  A/B matmuls are
                # interleaved so consecutive matmuls target opposite array
                # row-halves: LDWEIGHTS for one half pulls ahead under the
                # other half's running matmul instead of serializing.
                for g in range(4):
                    foff = (g // 2) * D2 + (g % 2) * GW
                    psA = pp.tile([B, GW], _f32, tag="ps")
                    psB = pp.tile([B, GW], _f32, tag="ps")
                    for j in range(GW // MM_N):
                        s = foff + j * MM_N
                        nc.tensor.matmul(
                            psA[:, j * MM_N:(j + 1) * MM_N],
                            cfgT2[0:E, :],
                            kt[0:E, s:s + MM_N],
                            start=True, stop=True,
                        )
                        nc.tensor.matmul(
                            psB[:, j * MM_N:(j + 1) * MM_N],
                            cfgT2[E:2 * E, :],
                            kt[E:2 * E, s:s + MM_N],
                            start=True, stop=True,
                        )
                    osl = slice(foff, foff + GW)
                    if (t + g) % 2 == 0:
                        nc.vector.tensor_copy(otA[:, osl], psA[:])
                        nc.scalar.copy(otB[:, osl], psB[:])
                    else:
                        nc.scalar.copy(otA[:, osl], psA[:])
                        nc.vector.tensor_copy(otB[:, osl], psB[:])
                for ot in (otA, otB):
                    # ~24 MB per HWDGE ring: every 4th output via SP
                    eng = nc.sync if out_dma_i % 4 == 3 else nc.scalar
                    eng.dma_start(out_ap[out_dma_i], ot[:])
                    out_dma_i += 1
    _split_multi_waits(nc)
    return nc


_NC_CACHE = None


def _get_nc():
    global _NC_CACHE
    if _NC_CACHE is None:
        _NC_CACHE = _build()
    return _NC_CACHE


def kernel(config, kernel):
    global LAST_RESULT
    config = np.ascontiguousarray(np.asarray(config, dtype=np.float32))
    ktab = np.asarray(kernel, dtype=np.float32).reshape(E, D1, D2)

    in_maps = []
    for c in range(N_CORES):
        # this core's D1 rows as 16 tiles of [128, 4096] fp16:
        # tile t, partitions 0:64  = experts for D1-rows (4t, 4t+1),
        #         partitions 64:128 = experts for D1-rows (4t+2, 4t+3),
        # free 0:2048 = first row of the pair, 2048:4096 = second.
        ksl = ktab[:, c * D1_SH:(c + 1) * D1_SH, :].astype(ml_dtypes.bfloat16)
        ksl = np.ascontiguousarray(
            ksl.reshape(E, N_TILES, 2, 2, D2)
            .transpose(1, 2, 0, 3, 4)
            .reshape(N_TILES, B, TW)
        )
        in_maps.append({"config": config, "kslice": ksl})

    nc = _get_nc()
    res = bass_utils.run_bass_kernel_spmd(
        nc,
        in_maps,
        list(range(N_CORES)),
        trace=_TRACE,
        **_TRACE_KWARGS,
    )
    LAST_RESULT = res

    out = np.empty((B, D1, D2), dtype=np.float32)
    for c in range(N_CORES):
        # out dram row u = 2t + h, free j*2048 + d2  ->  D1-row 4t + 2h + j
        o = res.results[c]["out"].reshape(N_TILES, 2, B, 2, D2)
        o = o.transpose(2, 0, 1, 3, 4).reshape(B, D1_SH, D2)
        out[:, c * D1_SH:(c + 1) * D1_SH, :] = o.astype(np.float32)
    return out


# revision 14
# speedup vs baseline: 1.1541x; 1.1541x over previous
"""Trainium2 Bass kernel for nn_EnsembleSpace (moe_routing).

Reference computation (B=128, E=64, D1=512, D2=2048):
    idx  = top_k(config, 8)                     # [B, E] routing logits
    cfg  = softmax(config * topk_mask)          # full-width softmax
    cfg  = where(cfg < 1e-4, 0, cfg)
    out  = cfg @ kernel.reshape(E, D1*D2)       # [B, D1*D2] -> [B, D1, D2]

The problem is memory-bound: the expert table is read once and the output
written once, and the per-core HBM limit is ~358 GB/s.  Two levers:

1. D1-sharding over the 8 cores (each core reads 1/8 of the table and
   writes 1/8 of the output, no collective).
2. 16-bit streaming: the table slice is pre-cast to bf16 on the host
   while laying out the shards, and the output is written to HBM as fp16
   and upcast on the host during the gather.  This halves per-core
   traffic from 96 MB to 48 MB.  Matmuls run bf16 x bf16 -> fp32 PSUM
   (bf16 streams 1 column/cycle on the PE; fp16 risks the half-rate
   mantissa path), so the only precision loss is quantization of the
   table / weights / output (~4e-3 rel err, inside the 2e-2 gate with
   4.6x margin).

Each core:
  1. computes the routing weights cfg [128, 64] on-chip (iterative top-8
     via 7 max+knockout rounds, exp+sum via one ACT op, eps mask),
  2. transposes cfg to [E, B] via two col-tiled identity matmuls so the
     weights land in BOTH partition halves (rows 0-63 and 64-127), then
     converts them to bf16,
  3. streams its table slice as 16 tiles of [128, 4096] bf16 (8 KB per
     partition line, full SBUF-port rate); each tile holds 4 D1-rows
     (two in each partition half); matmuls (N=512, bf16) alternate
     between the two array row-halves every instruction so each
     LDWEIGHTS targets the row-group the running matmul does NOT occupy
     and pulls ahead (same-half back-to-back matmuls serialize
     LDW+fill+drain at ~750 ns; alternated they stream at ~1 col/cycle).
     Pairs of [128, 1024] 2-bank PSUM tiles (4 bufs) drain via
     PSUM->SBUF copies (alternating DVE / ACT) that downconvert into
     fp16 out tiles; 1 MB fp16 out DMAs per D1-row pair.

Input DMAs and 1/4 of output DMAs ride the SP HWDGE ring, the other
output DMAs the ACT ring (~24 MB per ring); the 16 SDMA engines drain
both rings round-robin at packet granularity.
"""

import sys

for _p in ("/opt/trn_rl_repo", "/root/.axon_site/_ro/trn_rl_repo"):
    if _p not in sys.path:
        sys.path.append(_p)

import ml_dtypes
import numpy as np
import concourse.bass as bass
from concourse import tile, masks, bass_utils

mybir = bass.mybir
_f32 = mybir.dt.float32
_f16 = mybir.dt.float16
_bf16 = mybir.dt.bfloat16
_X = mybir.AxisListType.X
_alu = mybir.AluOpType

B, E, D1, D2 = 128, 64, 512, 2048
N_CORES = 8
D1_SH = D1 // N_CORES          # 64 D1-rows per core
ROWS_PER_TILE = 4              # D1-rows per [128, 4096] bf16 input tile
N_TILES = D1_SH // ROWS_PER_TILE   # 16
TW = 2 * D2                    # tile free size: 4096 x 16-bit = 8 KB/partition
MM_N = 512                     # one bf16 matmul per PSUM-bank-aligned slice
TOP_K = 8
SPARSE_EPS = 1e-4

_TRACE = False                 # test.py flips this for profiled runs
_TRACE_KWARGS = {}
LAST_RESULT = None             # BassKernelResults of the last run


def _split_multi_waits(nc):
    """This walrus build rejects >1 sync-wait per instruction.  Tile's
    add_semaphores emits multi-wait instructions (and the kernel-tail drain
    waits on every live semaphore).  Move the extra waits onto same-engine
    nops inserted immediately before the instruction — the engine executes
    serially, so blocking on the nops is equivalent."""
    n_split = 0
    for bb in nc.m.functions[0].blocks:
        out = []
        changed = False
        for inst in bb.instructions:
            si = inst.sync_info
            waits = list(si.on_wait) if (si is not None and si.on_wait) else []
            if len(waits) > 1:
                changed = True
                for w in waits[:-1]:
                    n_split += 1
                    nop = mybir.InstNoOp(name=f"I-waitsplit-{n_split}")
                    nop.engine = inst.engine
                    nop.sync_info = mybir.SyncInfo(on_wait=[w], on_update=[])
                    out.append(nop)
                inst.sync_info = mybir.SyncInfo(
                    on_wait=[waits[-1]], on_update=list(si.on_update or [])
                )
            out.append(inst)
        if changed:
            bb.instructions = out


def _routing_weights(nc, rp, pp, cfg_ap):
    """cfg [B, E] -> cfgT [E, B] bf16 in SBUF (top-8 mask, softmax, eps)."""
    cfgin = rp.tile([B, E], _f32, tag="cfgin")
    nc.sync.dma_start(cfgin[:], cfg_ap[:])

    # 8th-largest per row, in exp-space: exp(config) is positive and
    # order-preserving, so "knock out the max" is a 2-op zero-replace
    # (zero can never shadow a remaining value) instead of a 3-op -inf add
    e0 = rp.tile([B, E], _f32, tag="e0")
    nc.scalar.activation(e0[:], cfgin[:], mybir.ActivationFunctionType.Exp)
    t = rp.tile([B, E], _f32, tag="t")
    nc.vector.tensor_copy(t[:], e0[:])
    mk = rp.tile([B, 1], _f32, tag="mk")
    for _ in range(TOP_K - 1):
        nc.vector.reduce_max(mk[:], t[:], axis=_X)
        nc.vector.scalar_tensor_tensor(
            t[:], t[:], mk[:], t[:], op0=_alu.is_lt, op1=_alu.mult
        )
    m8 = rp.tile([B, 1], _f32, tag="m8")
    nc.vector.reduce_max(m8[:], t[:], axis=_X)

    # cfg0 = (exp(config) >= exp(m8)) * config ; softmax ; eps mask
    cfg0 = rp.tile([B, E], _f32, tag="cfg0")
    nc.vector.scalar_tensor_tensor(
        cfg0[:], e0[:], m8[:], cfgin[:], op0=_alu.is_ge, op1=_alu.mult
    )
    ecfg = rp.tile([B, E], _f32, tag="ecfg")
    zs = rp.tile([B, 1], _f32, tag="zs")
    nc.scalar.activation(
        ecfg[:], cfg0[:], mybir.ActivationFunctionType.Exp, accum_out=zs[:]
    )
    rz = rp.tile([B, 1], _f32, tag="rz")
    nc.vector.reciprocal(rz[:], zs[:])
    cfgn = rp.tile([B, E], _f32, tag="cfgn")
    nc.vector.tensor_scalar_mul(cfgn[:], ecfg[:], rz[:])
    cfgf = rp.tile([B, E], _f32, tag="cfgf")
    nc.vector.scalar_tensor_tensor(
        cfgf[:], cfgn[:], SPARSE_EPS, cfgn[:], op0=_alu.is_ge, op1=_alu.mult
    )

    # transpose to [E, B], replicated into both partition halves so the
    # row-packed matmuls can source weights at array rows 0-63 and 64-127;
    # the PSUM->SBUF copy downconverts to bf16 for the bf16 matmuls
    ident = rp.tile([B, B], _f32, tag="ident")
    masks.make_identity(nc, ident[:])
    psT = pp.tile([B, B], _f32, tag="ps")
    nc.tensor.matmul(psT[0:E, :], cfgf[:], ident[:], start=True, stop=True)
    nc.tensor.matmul(psT[E:2 * E, :], cfgf[:], ident[:], start=True, stop=True)
    cfgT2 = rp.tile([B, B], _bf16, tag="cfgT2")
    nc.vector.tensor_copy(cfgT2[:], psT[:])
    return cfgT2


def _build():
    nc = bass.Bass(
        "TRN2", target_bir_lowering=False, debug=False, num_devices=N_CORES
    )
    cfg_ap = nc.dram_tensor("config", [B, E], _f32, kind="ExternalInput").ap()
    ks_ap = nc.dram_tensor(
        "kslice", [N_TILES, B, TW], _bf16, kind="ExternalInput"
    ).ap()
    out_ap = nc.dram_tensor(
        "out", [2 * N_TILES, B, TW], _f16, kind="ExternalOutput"
    ).ap()

    with tile.TileContext(nc) as tc:
        with tc.tile_pool(name="route", bufs=1) as rp, \
             tc.tile_pool(name="inp", bufs=12) as ip, \
             tc.tile_pool(name="outp", bufs=10) as op_, \
             tc.tile_pool(name="ps", bufs=4, space="PSUM") as pp:
            cfgT2 = _routing_weights(nc, rp, pp, cfg_ap)
            out_dma_i = 0
            GW = 1024              # psum group width: 2 banks
            # Input DMAs are prefetched PF tiles ahead of the compute
            # wavefront.  Emitting kt_{t+PF} BEFORE tile t's output DMAs
            # matters: the SP engine FIFO is strictly ordered, and an
            # output dma_start waits on its tile's copies — a kt issued
            # after it would inherit that wait and cap the input prefetch
            # at one tile per compute period no matter how deep the pool.
            PF = 4
            kts = [
                ip.tile([B, TW], _bf16, tag="kt", name=f"kt{t}")
                for t in range(N_TILES)
            ]
            for t in range(PF):
                nc.sync.dma_start(kts[t][:], ks_ap[t])
            for t in range(N_TILES):
                kt = kts[t]
                if t + PF < N_TILES:
                    nc.sync.dma_start(kts[t + PF][:], ks_ap[t + PF])
                otA = op_.tile([B, TW], _f16, tag="ot")
                otB = op_.tile([B, TW], _f16, tag="ot")
                # 4 psum groups per tile; group g covers free slice
                # [foff, foff+1024) of BOTH out tiles.  A/B matmuls are
                # interleaved so consecutive matmuls target opposite array
                # row-halves: LDWEIGHTS for one half pulls ahead under the
                # other half's running matmul instead of serializing.
                for g in range(4):
                    foff = (g // 2) * D2 + (g % 2) * GW
                    psA = pp.tile([B, GW], _f32, tag="ps")
                    psB = pp.tile([B, GW], _f32, tag="ps")
                    for j in range(GW // MM_N):
                        s = foff + j * MM_N
                        nc.tensor.matmul(
                            psA[:, j * MM_N:(j + 1) * MM_N],
                            cfgT2[0:E, :],
                            kt[0:E, s:s + MM_N],
                            start=True, stop=True,
                        )
                        nc.tensor.matmul(
                            psB[:, j * MM_N:(j + 1) * MM_N],
                            cfgT2[E:2 * E, :],
                            kt[E:2 * E, s:s + MM_N],
                            start=True, stop=True,
                        )
                    osl = slice(foff, foff + GW)
                    if (t + g) % 2 == 0:
                        nc.vector.tensor_copy(otA[:, osl], psA[:])
                        nc.scalar.copy(otB[:, osl], psB[:])
                    else:
                        nc.scalar.copy(otA[:, osl], psA[:])
                        nc.vector.tensor_copy(otB[:, osl], psB[:])
                for ot in (otA, otB):
                    # ~24 MB per HWDGE ring: every 4th output via SP
                    eng = nc.sync if out_dma_i % 4 == 3 else nc.scalar
                    eng.dma_start(out_ap[out_dma_i], ot[:])
                    out_dma_i += 1
    _split_multi_waits(nc)
    return nc


_NC_CACHE = None


def _get_nc():
    global _NC_CACHE
    if _NC_CACHE is None:
        _NC_CACHE = _build()
    return _NC_CACHE


def kernel(config, kernel):
    global LAST_RESULT
    config = np.ascontiguousarray(np.asarray(config, dtype=np.float32))
    ktab = np.asarray(kernel, dtype=np.float32).reshape(E, D1, D2)

    in_maps = []
    for c in range(N_CORES):
        # this core's D1 rows as 16 tiles of [128, 4096] fp16:
        # tile t, partitions 0:64  = experts for D1-rows (4t, 4t+1),
        #         partitions 64:128 = experts for D1-rows (4t+2, 4t+3),
        # free 0:2048 = first row of the pair, 2048:4096 = second.
        ksl = ktab[:, c * D1_SH:(c + 1) * D1_SH, :].astype(ml_dtypes.bfloat16)
        ksl = np.ascontiguousarray(
            ksl.reshape(E, N_TILES, 2, 2, D2)
            .transpose(1, 2, 0, 3, 4)
            .reshape(N_TILES, B, TW)
        )
        in_maps.append({"config": config, "kslice": ksl})

    nc = _get_nc()
    res = bass_utils.run_bass_kernel_spmd(
        nc,
        in_maps,
        list(range(N_CORES)),
        trace=_TRACE,
        **_TRACE_KWARGS,
    )
    LAST_RESULT = res

    out = np.empty((B, D1, D2), dtype=np.float32)
    for c in range(N_CORES):
        # out dram row u = 2t + h, free j*2048 + d2  ->  D1-row 4t + 2h + j
        o = res.results[c]["out"].reshape(N_TILES, 2, B, 2, D2)
        o = o.transpose(2, 0, 1, 3, 4).reshape(B, D1_SH, D2)
        out[:, c * D1_SH:(c + 1) * D1_SH, :] = o.astype(np.float32)
    return out
